# revision 4
# baseline (speedup 1.0000x reference)
"""Multi-head causal self-attention (B=4, T=1024, d_model=2048, 16 heads of 128)
for 8 Trainium2 NeuronCores.

Sharding: hybrid data x tensor parallel. Core c handles batch b = c//2 and
head group g = c%2 (8 heads per core). Each core computes q/k/v projections
for its 8 heads, causal flash-style attention, and the out-projection rows
for those heads, producing a partial [1024, 2048] output (fp16) for its
batch. The host sums the two partials per batch and adds the output bias.

v2 changes over the first working version:
  - DMA: batched into ~512KB triggers spread across the sync/scalar/gpsimd
    queues in consumption order (trigger issue was 690ns each on one queue,
    starving the k-projection mid-phase and re-throttling the PE).
  - Attention: S^T blocks for a whole (head, q-chunk) pass are packed into
    one multi-bank PSUM tile (tail blocks share a bank column-wise) and
    exp'd with ONE ACTIVATE per pass. ACTIVATE costs (N+352)/1.2ns, so the
    per-instruction overhead was making the Scalar engine the attention
    bottleneck (85% busy). Passes are software-pipelined one ahead so the
    exp latency hides under the S/AV/den matmuls of neighbouring passes.
  - PSUM: one shared 3-bank tag (bufs=2) serves q/k projection accumulator
    pairs, attention S passes, and out-projection n-groups; plus 1-bank
    att/den accumulators = exactly 8 banks.
  - Output partials are written fp16 (halves the output-DMA tail).

All on-device layouts are feature-major so no transposes are needed:
  - x is shipped pre-transposed per batch: xt [2048, 1024] (fp16)
  - q, k are produced feature-major [dh, T] per head; v token-major [T, dh]
  - S^T[kv, q] = kf.T @ qf ; softmax denominator via ones[128,128] matmul
  - attention output accumulates as out^T[dh, q] = v_tm.T @ exp(S^T)
  - out^T is exactly the lhsT the out-projection needs
"""

import numpy as np

B, T, C = 4, 1024, 2048
H = 16          # total heads
HL = 8          # heads per core (local)
HB = 4          # heads per block
DH = 128        # head dim
KC = C // 128   # contraction chunks (16)
P = 128
NCORES = 8
BW = HB * DH    # head-block feature width (512)

_cache = {}


def _attn_passes(qc):
    """Pass layout for one (head, q-chunk): list of passes; each pass is a
    list of (bank, col_off, j, n, c0, diag) packed into one <=3-bank PSUM
    slot. j = kv block index, n = #q columns computed, c0 = q-col offset
    within the 512-wide q window, diag = needs causal mask on first 128
    cols. Used prefix of the slot is contiguous for one batched exp."""
    if qc == 0:
        # j: n = 512, 384, 256, 128 ; all diagonal-containing
        return [
            [(0, 0, 0, 512, 0, True),
             (1, 0, 1, 384, 128, True),
             (1, 384, 3, 128, 384, True),
             (2, 0, 2, 256, 256, True)],
        ]
    else:
        return [
            [(0, 0, 0, 512, 0, False),
             (1, 0, 1, 512, 0, False),
             (2, 0, 2, 512, 0, False)],
            [(0, 0, 3, 512, 0, False),
             (1, 0, 4, 512, 0, True),
             (2, 0, 5, 384, 128, True),
             (2, 384, 7, 128, 384, True)],
            [(0, 0, 6, 256, 256, True)],
        ]


def _pass_used_cols(pss):
    used = 0
    for bank, off, j, n, c0, diag in pss:
        used = max(used, bank * 512 + off + n)
    return used


def _build():
    import concourse.bacc as bacc
    import concourse.mybir as mybir
    import concourse.tile as tile

    F32 = mybir.dt.float32
    F16 = mybir.dt.float16
    AF = mybir.ActivationFunctionType
    ALU = mybir.AluOpType

    nc = bacc.Bacc("TRN2", target_bir_lowering=False, debug=False)

    xt_d = nc.dram_tensor("xt", (C, T), F16, kind="ExternalInput")
    wq_d = nc.dram_tensor("wq", (C, HL * DH), F16, kind="ExternalInput")
    wk_d = nc.dram_tensor("wk", (C, HL * DH), F16, kind="ExternalInput")
    wv_d = nc.dram_tensor("wv", (C, HL * DH), F16, kind="ExternalInput")
    wo_d = nc.dram_tensor("wo", (HL * DH, C), F16, kind="ExternalInput")
    bq_d = nc.dram_tensor("bq", (P, HL), F32, kind="ExternalInput")
    bk_d = nc.dram_tensor("bk", (P, HL), F32, kind="ExternalInput")
    bvb_d = nc.dram_tensor("bvb", (P, HL * DH), F32, kind="ExternalInput")
    mask_d = nc.dram_tensor("mask", (P, P), F32, kind="ExternalInput")
    part_d = nc.dram_tensor("part", (T, C), F16, kind="ExternalOutput")

    # grouped views for batched DMA (4 kc per transfer)
    xt_v = xt_d.rearrange("(g k p) t -> p g k t", p=P, k=2)       # 8 groups of 2
    wq_v = wq_d.rearrange("(g k p) m -> p g k m", p=P, k=4)       # 4 groups of 4
    wk_v = wk_d.rearrange("(g k p) m -> p g k m", p=P, k=4)
    wv_v = wv_d.rearrange("(g k p) m -> p g k m", p=P, k=4)
    wo_v = wo_d.rearrange("(h p) n -> p h n", p=P)
    part_v = part_d.rearrange("(mo p) n -> p mo n", p=P)

    with tile.TileContext(nc) as tc:
        with (
            tc.tile_pool(name="res", bufs=1) as res,
            tc.tile_pool(name="wblk", bufs=1) as wblk,
            tc.tile_pool(name="qkv", bufs=2) as qkv,
            tc.tile_pool(name="wp", bufs=3) as wp,
            tc.tile_pool(name="ps", bufs=1, space="PSUM") as ps,
        ):
            bq_sb = res.tile([P, HL], F32, tag="bq")
            bk_sb = res.tile([P, HL], F32, tag="bk")
            bvb_sb = res.tile([P, HL * DH], F32, tag="bvb")
            mask_sb = res.tile([P, P], F32, tag="mask")

            ones_sb = res.tile([P, P], F16, tag="ones")
            nc.vector.memset(ones_sb[:], 1.0)

            # x^T in 8 chunks of 2 kc; first chunk halved for earlier start
            xts = [res.tile([P, 2, T], F16, tag=f"xt{g}", name=f"xt{g}")
                   for g in range(8)]

            def xt_ap(kc):
                return xts[kc // 2][:, kc % 2, :]

            # weight chunks: 4 groups of 4 kc per (wname); reused across blocks
            wts = {w: [wblk.tile([P, 4, BW], F16, tag=f"{w}{g}", name=f"{w}{g}")
                       for g in range(4)] for w in ("wq", "wk", "wv")}

            def w_ap(wname, kc):
                return wts[wname][kc // 4][:, kc % 4, :]

            wo_sb = res.tile([P, HL, C], F16, tag="wo")
            oT = res.tile([P, HL, T], F16, tag="oT")

            # ---------------- DMA: block 0 inputs, consumption-ordered,
            # spread across the three trigger-capable queues ----------------
            # sync: x^T (first chunk split in half for an earlier first MM)
            nc.sync.dma_start(xts[0][:, 0, :], xt_v[:, 0, 0, :])
            nc.sync.dma_start(xts[0][:, 1, :], xt_v[:, 0, 1, :])
            for g in range(1, 8):
                nc.sync.dma_start(xts[g][:], xt_v[:, g, :, :])
            # scalar: wq then wk (block 0)
            nc.scalar.dma_start(wts["wq"][0][:, 0, :], wq_v[:, 0, 0, 0:BW])
            nc.scalar.dma_start(
                wts["wq"][0][:, 1:4, :], wq_v[:, 0, 1:4, 0:BW]
            )
            for g in range(1, 4):
                nc.scalar.dma_start(wts["wq"][g][:], wq_v[:, g, :, 0:BW])
            for g in range(4):
                nc.scalar.dma_start(wts["wk"][g][:], wk_v[:, g, :, 0:BW])
            # gpsimd: biases/mask, then wv (block 0)
            nc.gpsimd.dma_start(bq_sb[:], bq_d[:])
            nc.gpsimd.dma_start(bk_sb[:], bk_d[:])
            nc.gpsimd.dma_start(bvb_sb[:], bvb_d[:])
            nc.gpsimd.dma_start(mask_sb[:], mask_d[:])
            for g in range(4):
                nc.gpsimd.dma_start(wts["wv"][g][:], wv_v[:, g, :, 0:BW])

            # Warm the PE (HAM un-throttles after ~3.4us of activity) while
            # the first input DMAs stream in.
            warm = ps.tile([P, 3, 512], F32, tag="big", bufs=2)
            for i in range(64):
                nc.tensor.matmul(
                    warm[:, i % 3, 0:P], ones_sb[:], ones_sb[:],
                    start=True, stop=True,
                )

            qf = {}
            kf = {}
            vt = {}

            def proj_qk(blk):
                lo = blk * BW
                qf[blk] = qkv.tile([P, HB, T], F16, tag="qf", name=f"qf{blk}")
                kf[blk] = qkv.tile([P, HB, T], F16, tag="kf", name=f"kf{blk}")
                for dst, wname, bsb in (
                    (qf[blk], "wq", bq_sb), (kf[blk], "wk", bk_sb)
                ):
                    for h in range(HB):
                        pt = ps.tile([P, 3, 512], F32, tag="big", bufs=2)
                        for kc in range(KC):
                            w_sl = w_ap(wname, kc)[:, h * DH:(h + 1) * DH]
                            for t in range(2):
                                nc.tensor.matmul(
                                    pt[:, t, :],
                                    w_sl,
                                    xt_ap(kc)[:, t * 512:(t + 1) * 512],
                                    start=(kc == 0),
                                    stop=(kc == KC - 1),
                                )
                        for t in range(2):
                            nc.vector.tensor_tensor(
                                dst[:, h, t * 512:(t + 1) * 512],
                                pt[:, t, :],
                                bsb[
                                    :, blk * HB + h: blk * HB + h + 1
                                ].to_broadcast((P, 512)),
                                ALU.add,
                            )

            def proj_v(blk):
                lo = blk * BW
                vt[blk] = qkv.tile([P, T // P, BW], F16, tag="vt", name=f"vt{blk}")
                for m in range(T // P):
                    pt = ps.tile([P, 3, 512], F32, tag="big", bufs=2)
                    for kc in range(KC):
                        nc.tensor.matmul(
                            pt[:, 0, :],
                            xt_ap(kc)[:, m * P:(m + 1) * P],
                            w_ap("wv", kc),
                            start=(kc == 0),
                            stop=(kc == KC - 1),
                        )
                    nc.vector.tensor_tensor(
                        vt[blk][:, m, :], pt[:, 0, :],
                        bvb_sb[:, lo:lo + BW], ALU.add,
                    )

            # ---------------- attention: pipelined passes ----------------
            def attn_block(blk):
                """Emit causal attention for the 4 heads of `blk`, passes
                software-pipelined one ahead (S of pass i+1 issues before
                AV/den of pass i so the batched exp hides)."""
                work = []  # (l, qc, passes, pass_idx)
                for l in range(HB):
                    for qc in range(2):
                        pss = _attn_passes(qc)
                        for pi in range(len(pss)):
                            work.append((l, qc, pss, pi))

                state = {}  # (l, qc) -> dict with psum tiles

                def emit_S(l, qc, pss, pi):
                    sgrp = ps.tile(
                        [P, 3, 512], F32, tag="big", bufs=2,
                        name=f"sg{blk}_{l}_{qc}_{pi}",
                    )
                    for bank, off, j, n, c0, diag in pss[pi]:
                        nc.tensor.matmul(
                            sgrp[:, bank, off:off + n],
                            kf[blk][:, l, j * P:(j + 1) * P],
                            qf[blk][:, l, qc * 512 + c0: qc * 512 + c0 + n],
                            start=(off == 0),
                            stop=(off + n == 512) or (bank * 512 + off + n
                                                      == _pass_used_cols(pss[pi])),
                            skip_group_check=True,
                        )
                        if diag:
                            nc.vector.tensor_tensor(
                                sgrp[:, bank, off:off + P],
                                sgrp[:, bank, off:off + P],
                                mask_sb[:], ALU.add,
                            )
                    used = _pass_used_cols(pss[pi])
                    E = wp.tile([P, 3, 512], F16, tag="E", bufs=4)
                    sflat = sgrp.rearrange("p a b -> p (a b)")
                    eflat = E.rearrange("p a b -> p (a b)")
                    nc.scalar.activation(eflat[:, :used], sflat[:, :used], AF.Exp)
                    return E

                def emit_AVden(l, qc, pss, pi, E):
                    st = state[(l, qc)]
                    first = pi == 0
                    last = pi == len(pss) - 1
                    for idx, (bank, off, j, n, c0, diag) in enumerate(pss[pi]):
                        nc.tensor.matmul(
                            st["att"][:, c0:512],
                            vt[blk][:, j, l * DH:(l + 1) * DH],
                            E[:, bank, off:off + n],
                            start=(first and idx == 0),
                            stop=(last and idx == len(pss[pi]) - 1),
                            skip_group_check=True,
                        )
                    for idx, (bank, off, j, n, c0, diag) in enumerate(pss[pi]):
                        nc.tensor.matmul(
                            st["den"][:, c0:512],
                            ones_sb[:],
                            E[:, bank, off:off + n],
                            start=(first and idx == 0),
                            stop=(last and idx == len(pss[pi]) - 1),
                            skip_group_check=True,
                        )
                    if last:
                        hh = blk * HB + l
                        rc = wp.tile([P, 512], F32, tag="rc")
                        nc.vector.reciprocal_approx_fast(rc[:], st["den"][:])
                        nc.vector.tensor_tensor(
                            oT[:, hh, qc * 512:(qc + 1) * 512],
                            st["att"][:], rc[:], ALU.mult,
                        )

                pending = None  # (l, qc, pss, pi, E)
                for (l, qc, pss, pi) in work:
                    if pi == 0:
                        state[(l, qc)] = {
                            "att": ps.tile([P, 512], F32, tag="att",
                                           name=f"att{blk}_{l}_{qc}"),
                            "den": ps.tile([P, 512], F32, tag="den",
                                           name=f"den{blk}_{l}_{qc}"),
                        }
                    E = emit_S(l, qc, pss, pi)
                    if pending is not None:
                        emit_AVden(*pending)
                    pending = (l, qc, pss, pi, E)
                if pending is not None:
                    emit_AVden(*pending)

            # ---------------- block 0 ----------------
            proj_qk(0)
            proj_v(0)

            # block-1 weights: trigger after block-0 projections consume the
            # shared tiles (WAR deps serialize anyway); wo late on gpsimd.
            for g in range(4):
                nc.scalar.dma_start(wts["wq"][g][:], wq_v[:, g, :, BW:2 * BW])
            for g in range(4):
                nc.scalar.dma_start(wts["wk"][g][:], wk_v[:, g, :, BW:2 * BW])
            for g in range(4):
                nc.gpsimd.dma_start(wts["wv"][g][:], wv_v[:, g, :, BW:2 * BW])
            nc.gpsimd.dma_start(wo_sb[:], wo_v[:, :, :])

            attn_block(0)

            # ---------------- block 1 ----------------
            proj_qk(1)
            proj_v(1)
            attn_block(1)

            # ---------------- out projection ----------------
            for m in range(T // P):
                po = wp.tile([P, 4, 512], F16, tag="po", bufs=2)
                for ng, n2s in ((0, (0, 1, 2)), (1, (3,))):
                    pt = ps.tile([P, 3, 512], F32, tag="big", bufs=2,
                                 name=f"op{m}_{ng}")
                    for h in range(HL):
                        for bi, n2 in enumerate(n2s):
                            nc.tensor.matmul(
                                pt[:, bi, :],
                                oT[:, h, m * P:(m + 1) * P],
                                wo_sb[:, h, n2 * 512:(n2 + 1) * 512],
                                start=(h == 0),
                                stop=(h == HL - 1),
                            )
                    for bi, n2 in enumerate(n2s):
                        nc.vector.tensor_copy(po[:, n2, :], pt[:, bi, :])
                nc.sync.dma_start(part_v[:, m, :], po.rearrange("p a b -> p (a b)"))

    nc.compile()
    return nc


def _prep_inputs(x, w_qkv, b_qkv, w_out):
    """Build the 8 per-core input maps (host-side shard + layout prep)."""
    f16 = np.float16
    scale = np.float32(1.0 / np.sqrt(DH))

    xt = [np.ascontiguousarray(x[b].T).astype(f16) for b in range(B)]

    mask = np.where(
        np.arange(P)[None, :] >= np.arange(P)[:, None], 0.0, -1e30
    ).astype(np.float32)

    per_g = []
    for g in range(2):
        lo, hi = g * HL * DH, (g + 1) * HL * DH
        wq = np.ascontiguousarray(w_qkv[:, lo:hi] * scale).astype(f16)
        wk = np.ascontiguousarray(w_qkv[:, C + lo: C + hi]).astype(f16)
        wv = np.ascontiguousarray(w_qkv[:, 2 * C + lo: 2 * C + hi]).astype(f16)
        wo = np.ascontiguousarray(w_out[lo:hi, :]).astype(f16)
        bq = (b_qkv[lo:hi] * scale).astype(np.float32).reshape(HL, P).T.copy()
        bk = b_qkv[C + lo: C + hi].astype(np.float32).reshape(HL, P).T.copy()
        bv = b_qkv[2 * C + lo: 2 * C + hi].astype(np.float32)
        bvb = np.ascontiguousarray(np.broadcast_to(bv[None, :], (P, HL * DH)))
        per_g.append(dict(wq=wq, wk=wk, wv=wv, wo=wo, bq=bq, bk=bk, bvb=bvb))

    in_maps = []
    for c in range(NCORES):
        b, g = c // 2, c % 2
        m = dict(per_g[g])
        m["xt"] = xt[b]
        m["mask"] = mask
        in_maps.append(m)
    return in_maps


def run(x, w_qkv, b_qkv, w_out, b_out, trace=False, **trace_kwargs):
    from concourse.bass_utils import run_bass_kernel_spmd

    x = np.asarray(x, dtype=np.float32)
    w_qkv = np.asarray(w_qkv, dtype=np.float32)
    b_qkv = np.asarray(b_qkv, dtype=np.float32)
    w_out = np.asarray(w_out, dtype=np.float32)
    b_out = np.asarray(b_out, dtype=np.float32)

    if "nc" not in _cache:
        _cache["nc"] = _build()
    nc = _cache["nc"]

    in_maps = _prep_inputs(x, w_qkv, b_qkv, w_out)
    res = run_bass_kernel_spmd(
        nc, in_maps, core_ids=list(range(NCORES)), trace=trace, **trace_kwargs
    )

    out = np.empty((B, T, C), np.float32)
    for b in range(B):
        out[b] = (res.results[2 * b]["part"].astype(np.float32)
                  + res.results[2 * b + 1]["part"].astype(np.float32))
    out += b_out
    return out, res


def kernel(x, w_qkv, b_qkv, w_out, b_out):
    out, _ = run(x, w_qkv, b_qkv, w_out, b_out)
    return out
